# revision 13
# baseline (speedup 1.0000x reference)
"""Trainium2 Bass kernel: CrossAttention with T5-style relative position bias.

Sharding: 8 cores = (batch b in 0..3) x (head-group g in 0..1); each core
computes 8 heads of one batch. Wq/Wkv column-sharded per head, Wproj
row-sharded; the tiny Wproj partial-sum all-reduce is done on the host.

On-chip layout is fully "transposed" (contraction on partitions):
  qhT [o=512, n]   kT [o=512, m]   v [m, o=512] (+ ones col for denominators)
Scores are computed transposed (scoresT [m, n]) so the P @ V contraction
needs no transposes at all. Softmax is unnormalized exp (logits are O(3),
no max subtraction needed); the denominator comes from an extra ones
column appended to V; normalization is a cheap [64, 512] multiply.

The T5 bias b(m-n) is folded in as:
  exp(S/8 + b) = exp(S * 0.125 + c15 + (b - c15))
  - c15 = table[15] (the value for m - n <= -128) rides the ACT exp
    per-partition bias AP (free).
  - The residual 8*(b - c15) is nonzero only for m - n > -129, i.e. a
    prefix of each [m-chunk, n-row]; it is added by one DVE tensor_add
    per (head, m-chunk) from a host-precomputed Toeplitz table.
All matmuls run as float32r (full-rate fp32 path on the PE).
"""

import math
import numpy as np
from contextlib import ExitStack

import concourse.bacc as bacc
import concourse.bass as bass
import concourse.tile as tile
from concourse import mybir
from concourse.bass_utils import run_bass_kernel_spmd

B, N, M, C, H = 4, 1024, 1024, 1024, 16
D = C // H  # 64
HG = H // 2  # heads per core
NCORES = 8
NUM_BUCKETS, MAX_DISTANCE = 32, 128

F32 = mybir.dt.float32
F32R = mybir.dt.float32r
BF16 = mybir.dt.bfloat16
EXP = mybir.ActivationFunctionType.Exp

RES_W = 1152  # residual-bias table width: prefix 896 + band 256


def _build_program() -> bass.Bass:
    nc = bacc.Bacc()

    qT = nc.dram_tensor("qT", [C, N], F32R, kind="ExternalInput")
    kvT = nc.dram_tensor("kvT", [C, M], F32R, kind="ExternalInput")
    WqT = nc.dram_tensor("WqT", [C, 512], F32R, kind="ExternalInput")
    WkT = nc.dram_tensor("WkT", [C, 512], F32R, kind="ExternalInput")
    WvT = nc.dram_tensor("WvT", [C, 512], F32R, kind="ExternalInput")
    WpT = nc.dram_tensor("WpT", [512, C], F32R, kind="ExternalInput")
    res = nc.dram_tensor("res", [HG, 128, RES_W], BF16, kind="ExternalInput")
    c15 = nc.dram_tensor("c15", [128, HG], F32, kind="ExternalInput")

    outp = nc.dram_tensor("outp", [N, C], F32, kind="ExternalOutput")
    k_out = nc.dram_tensor("k_out", [HG, D, M], F32R, kind="ExternalOutput")
    v_out = nc.dram_tensor("v_out", [HG, M, D], F32R, kind="ExternalOutput")

    with tile.TileContext(nc) as tc, ExitStack() as ctx:
        keep = ctx.enter_context(tc.tile_pool(name="keep", bufs=1))
        qh_sb = keep.tile([128, 4, N], F32R)  # qhT: head-pair blocks x n
        kT_sb = keep.tile([128, 4, M], F32R)  # kT: head-pair blocks x m
        vA_sb = keep.tile([128, 8, HG, D + 1], F32R)  # v (+ ones col)
        ao_sb = keep.tile([128, 4, N], F32R)  # attn-out T (a x n)
        c15_sb = keep.tile([128, HG], F32)

        nc.sync.dma_start(out=c15_sb, in_=c15[:])
        nc.vector.memset(vA_sb[:, :, :, D].bitcast(F32), 1.0)

        # ---- Phase A: projections ----
        with ExitStack() as esA:
            pa = esA.enter_context(tc.tile_pool(name="pa", bufs=1))
            qT_sb = pa.tile([128, 8, N], F32R)
            kvT_sb = pa.tile([128, 8, M], F32R)
            wq_sb = pa.tile([128, 8, 512], F32R)
            wk_sb = pa.tile([128, 8, 512], F32R)
            wv_sb = pa.tile([128, 8, 512], F32R)
            pps = esA.enter_context(tc.tile_pool(name="pps", bufs=4, space="PSUM"))

            nc.sync.dma_start(out=wq_sb, in_=WqT.rearrange("(a p) o -> p a o", p=128))
            nc.sync.dma_start(out=wk_sb, in_=WkT.rearrange("(a p) o -> p a o", p=128))
            nc.sync.dma_start(out=wv_sb, in_=WvT.rearrange("(a p) o -> p a o", p=128))
            nc.sync.dma_start(out=qT_sb, in_=qT.rearrange("(a p) n -> p a n", p=128))
            nc.sync.dma_start(out=kvT_sb, in_=kvT.rearrange("(a p) n -> p a n", p=128))

            # qhT / kT: out[o, n] = W.T[o, c] @ xT[c, n]
            for x_sb, w_sb, o_sb in ((qT_sb, wq_sb, qh_sb), (kvT_sb, wk_sb, kT_sb)):
                for pb in range(4):
                    for nh in range(2):
                        ps = pps.tile([128, 512], F32, tag="pa_ps")
                        for ci in range(8):
                            nc.tensor.matmul(
                                ps,
                                (w_sb[:, ci, pb * 128 : (pb + 1) * 128]),
                                (x_sb[:, ci, nh * 512 : (nh + 1) * 512]),
                                start=(ci == 0),
                                stop=(ci == 7),
                            )
                        nc.scalar.copy(
                            out=o_sb[:, pb, nh * 512 : (nh + 1) * 512], in_=ps
                        )

            # v: out[m, o] = kv[m, c] @ Wv.T[c, o]
            for mb in range(8):
                ps = pps.tile([128, 512], F32, tag="pa_ps")
                for ci in range(8):
                    nc.tensor.matmul(
                        ps,
                        (kvT_sb[:, ci, mb * 128 : (mb + 1) * 128]),
                        (wv_sb[:, ci, :]),
                        start=(ci == 0),
                        stop=(ci == 7),
                    )
                nc.vector.tensor_copy(
                    out=vA_sb[:, mb, :, 0:D],
                    in_=ps[:].rearrange("p (h d) -> p h d", h=HG),
                )
                nc.sync.dma_start(
                    out=v_out[:, mb * 128 : (mb + 1) * 128, :].rearrange(
                        "h m d -> m h d"
                    ),
                    in_=vA_sb[:, mb, :, 0:D],
                )

            for pb in range(4):
                nc.sync.dma_start(
                    out=k_out[2 * pb : 2 * pb + 2].rearrange("t d m -> (t d) m"),
                    in_=kT_sb[:, pb, :],
                )

        # ---- Phase B: attention ----
        with ExitStack() as esB:
            pbp = esB.enter_context(tc.tile_pool(name="pbp", bufs=1))
            wp_sb = pbp.tile([128, 4, C], F32R)
            nc.sync.dma_start(out=wp_sb, in_=WpT.rearrange("(a p) o -> p a o", p=128))
            res_pool = esB.enter_context(tc.tile_pool(name="res", bufs=3))
            pt_pool = esB.enter_context(tc.tile_pool(name="pt", bufs=3))
            nrm_pool = esB.enter_context(tc.tile_pool(name="nrm", bufs=4))
            dbounce = esB.enter_context(tc.tile_pool(name="dbounce", bufs=4, space="DRAM"))

            with ExitStack() as esB2:
                sc_pool = esB2.enter_context(
                    tc.tile_pool(name="sc", bufs=2, space="PSUM")
                )
                o_pool = esB2.enter_context(
                    tc.tile_pool(name="op", bufs=4, space="PSUM")
                )

                for pr in range(4):
                    res_t = []
                    for hi in range(2):
                        rt = res_pool.tile([128, RES_W], BF16, tag="res")
                        nc.sync.dma_start(out=rt, in_=res[2 * pr + hi])
                        res_t.append(rt)
                    ops = {
                        (hi, nh): o_pool.tile(
                            [65, 512], F32, tag="o", name=f"o_{pr}_{hi}_{nh}"
                        )
                        for hi in range(2)
                        for nh in range(2)
                    }
                    for mi in range(8):
                        m0 = mi * 128
                        for hi in range(2):
                            h = 2 * pr + hi
                            st = sc_pool.tile([128, 1024], F32, tag="sc")
                            for nh in range(2):
                                nc.tensor.matmul(
                                    st[:, nh * 512 : (nh + 1) * 512],
                                    (kT_sb[hi * 64 : hi * 64 + 64, pr, m0 : m0 + 128]),
                                    (
                                        qh_sb[
                                            hi * 64 : hi * 64 + 64,
                                            pr,
                                            nh * 512 : (nh + 1) * 512,
                                        ]
                                    ),
                                    start=True,
                                    stop=True,
                                    tile_position=(hi * 64, 0),
                                )
                            L = min(m0 + 256, 1024)
                            nc.vector.tensor_add(
                                st[:, 0:L],
                                st[:, 0:L],
                                res_t[hi][:, 896 - m0 : 896 - m0 + L],
                            )
                            pt = pt_pool.tile([128, 1024], F32R, tag="pt")
                            nc.scalar.activation(
                                out=pt,
                                in_=st[:],
                                func=EXP,
                                bias=c15_sb[:, h : h + 1],
                                scale=0.125,
                            )
                            for nh in range(2):
                                nc.tensor.matmul(
                                    ops[(hi, nh)],
                                    (vA_sb[:, mi, h, :]),
                                    (pt[:, nh * 512 : (nh + 1) * 512]),
                                    start=(mi == 0),
                                    stop=(mi == 7),
                                    skip_group_check=True,
                                )
                    # normalize: rows 0..63 = sum(exp * v), row 64 = sum(exp)
                    for hi in range(2):
                        for nh in range(2):
                            op = ops[(hi, nh)]
                            bc = nrm_pool.tile([65, 512], F32, tag="bc")
                            nc.vector.reciprocal(out=bc[64:65, :], in_=op[64:65, :])
                            db = dbounce.tile([1, 512], F32, tag="db")
                            nc.sync.dma_start(out=db, in_=bc[64:65, :])
                            nc.sync.dma_start(
                                out=bc[0:64, :], in_=db.to_broadcast((64, 512))
                            )
                            if hi == 0:
                                nc.vector.tensor_mul(
                                    ao_sb[0:64, pr, nh * 512 : (nh + 1) * 512],
                                    op[0:64, :],
                                    bc[0:64, :],
                                )
                            else:
                                sr = nrm_pool.tile([64, 512], F32R, tag="sr")
                                nc.vector.tensor_mul(sr, op[0:64, :], bc[0:64, :])
                                nc.sync.dma_start(
                                    out=ao_sb[64:128, pr, nh * 512 : (nh + 1) * 512],
                                    in_=sr,
                                )

            # ---- Phase C: output projection (partial; host adds halves+bias)
            with ExitStack() as esC:
                pcs = esC.enter_context(tc.tile_pool(name="pcs", bufs=2, space="PSUM"))
                out_pool = esC.enter_context(tc.tile_pool(name="out", bufs=2))
                for ni in range(8):
                    ot = out_pool.tile([128, 1024], F32, tag="ot")
                    for oh in range(2):
                        ps = pcs.tile([128, 512], F32, tag="cps")
                        for ab in range(4):
                            nc.tensor.matmul(
                                ps,
                                (ao_sb[:, ab, ni * 128 : (ni + 1) * 128]),
                                (wp_sb[:, ab, oh * 512 : (oh + 1) * 512]),
                                start=(ab == 0),
                                stop=(ab == 3),
                            )
                        nc.scalar.copy(out=ot[:, oh * 512 : (oh + 1) * 512], in_=ps)
                    nc.sync.dma_start(out=outp[ni * 128 : (ni + 1) * 128, :], in_=ot)

    nc.compile()
    return nc


def _round_f32r(x):
    """Round f32 to the fp32r (TF32-like, 11-bit mantissa) grid, matching
    walrus's fp32_to_fp32r (round-half-up at bit 12)."""
    u = np.ascontiguousarray(x, np.float32).view(np.uint32)
    return ((u + np.uint32(0x800)) & np.uint32(0xFFFFF000)).view(np.float32)


def _bucket_np(r):
    """numpy mirror of reference.relative_position_bucket (f32 arithmetic)."""
    nb = NUM_BUCKETS // 2  # 16
    rb = (r > 0).astype(np.int32) * nb
    ar = np.abs(r)
    me = nb // 2  # 8
    is_small = ar < me
    rf = np.maximum(ar, 1).astype(np.float32)
    large = me + (
        np.log(rf / np.float32(me))
        / np.float32(math.log(MAX_DISTANCE / me))
        * np.float32(nb - me)
    ).astype(np.int32)
    large = np.minimum(large, nb - 1)
    return rb + np.where(is_small, ar.astype(np.int32), large)


_CACHE = {}


def _program():
    if "nc" not in _CACHE:
        _CACHE["nc"] = _build_program()
    return _CACHE["nc"]


def _host_bias_tables(bias_table):
    """res[g][h, p, rc] = 8*(table[bucket(p + 896 - rc), H0+h] - c15) as bf16,
    plus c15 rows [128, HG] per group."""
    import ml_dtypes

    p = np.arange(128, dtype=np.int64)[:, None]
    rc = np.arange(RES_W, dtype=np.int64)[None, :]
    bkt = _bucket_np(p + 896 - rc)  # [128, RES_W]
    res_g, c15_g = [], []
    for g in range(2):
        H0 = g * HG
        tab = bias_table[:, H0 : H0 + HG].astype(np.float32)  # [32, HG]
        c15 = tab[15]  # [HG]
        r = 8.0 * (tab[bkt] - c15[None, None, :])  # [128, RES_W, HG]
        res_g.append(
            np.ascontiguousarray(r.transpose(2, 0, 1)).astype(ml_dtypes.bfloat16)
        )
        c15_g.append(np.ascontiguousarray(np.broadcast_to(c15, (128, HG))).astype(np.float32))
    return res_g, c15_g


def kernel(q, kv, Wq, Wkv, Wproj, bproj, bias_table):
    q = np.asarray(q, np.float32)
    kv = np.asarray(kv, np.float32)
    Wq = np.asarray(Wq, np.float32)
    Wkv = np.asarray(Wkv, np.float32)
    Wproj = np.asarray(Wproj, np.float32)
    bproj = np.asarray(bproj, np.float32)
    bias_table = np.asarray(bias_table, np.float32)

    nc = _program()
    res_g, c15_g = _host_bias_tables(bias_table)

    wq_g, wk_g, wv_g, wp_g = [], [], [], []
    for g in range(2):
        s = slice(512 * g, 512 * (g + 1))
        wq_g.append(_round_f32r(Wq[s, :].T))
        wk_g.append(_round_f32r(Wkv[s, :].T))
        wv_g.append(_round_f32r(Wkv[1024 + 512 * g : 1024 + 512 * (g + 1), :].T))
        wp_g.append(_round_f32r(Wproj[:, s].T))

    in_maps = []
    for core in range(NCORES):
        b, g = core // 2, core % 2
        in_maps.append(
            {
                "qT": _round_f32r(q[b].T),
                "kvT": _round_f32r(kv[b].T),
                "WqT": wq_g[g],
                "WkT": wk_g[g],
                "WvT": wv_g[g],
                "WpT": wp_g[g],
                "res": res_g[g],
                "c15": c15_g[g],
            }
        )

    results = run_bass_kernel_spmd(
        nc, in_maps, core_ids=list(range(NCORES)), **_CACHE.get("run_kwargs", {})
    )
    _CACHE["last_results"] = results

    out = np.zeros((B, N, C), np.float32)
    key = np.empty((B, H, M, D), np.float32)
    value = np.empty((B, H, M, D), np.float32)
    for core, r in enumerate(results.results):
        b, g = core // 2, core % 2
        H0 = g * HG
        out[b] += r["outp"]
        key[b, H0 : H0 + HG] = np.asarray(r["k_out"]).transpose(0, 2, 1)
        value[b, H0 : H0 + HG] = np.asarray(r["v_out"])
    out += bproj[None, None, :]
    return out, key, value


# revision 15
# speedup vs baseline: 1.0929x; 1.0929x over previous
"""Trainium2 Bass kernel: CrossAttention with T5-style relative position bias.

Sharding: 8 cores = (batch b in 0..3) x (head-group g in 0..1); each core
computes 8 heads of one batch. Wq/Wkv column-sharded per head, Wproj
row-sharded; the tiny Wproj partial-sum all-reduce is done on the host.

On-chip layout is fully "transposed" (contraction on partitions):
  qhT [o=512, n]   kT [o=512, m]   v [m, o=512] (+ ones col for denominators)
Scores are computed transposed (scoresT [m, n]) so the P @ V contraction
needs no transposes at all. Softmax is unnormalized exp (logits are O(3),
no max subtraction needed); the denominator comes from an extra ones
column appended to V; normalization is a cheap [64, 512] multiply.

The T5 bias b(m-n) is folded in as:
  exp(S/8 + b) = exp(S * 0.125 + c15 + (b - c15))
  - c15 = table[15] (the value for m - n <= -128) rides the ACT exp
    per-partition bias AP (free).
  - The residual 8*(b - c15) is nonzero only for m - n > -129, i.e. a
    prefix of each [m-chunk, n-row]; it is added by one DVE tensor_add
    per (head, m-chunk) from a host-precomputed Toeplitz table.
All matmuls run as float32r (full-rate fp32 path on the PE).
"""

import math
import numpy as np
from contextlib import ExitStack

import concourse.bacc as bacc
import concourse.bass as bass
import concourse.tile as tile
from concourse import mybir
from concourse.bass_utils import run_bass_kernel_spmd

B, N, M, C, H = 4, 1024, 1024, 1024, 16
D = C // H  # 64
HG = H // 2  # heads per core
NCORES = 8
NUM_BUCKETS, MAX_DISTANCE = 32, 128

F32 = mybir.dt.float32
F32R = mybir.dt.float32r
BF16 = mybir.dt.bfloat16
EXP = mybir.ActivationFunctionType.Exp

RES_W = 1152  # residual-bias table width: prefix 896 + band 256


def _build_program() -> bass.Bass:
    nc = bacc.Bacc()

    qT = nc.dram_tensor("qT", [C, N], BF16, kind="ExternalInput")
    kvT = nc.dram_tensor("kvT", [C, M], F32R, kind="ExternalInput")
    WqT = nc.dram_tensor("WqT", [C, 512], BF16, kind="ExternalInput")
    WkT = nc.dram_tensor("WkT", [C, 512], F32R, kind="ExternalInput")
    WvT = nc.dram_tensor("WvT", [C, 512], F32R, kind="ExternalInput")
    WpT = nc.dram_tensor("WpT", [512, C], BF16, kind="ExternalInput")
    res = nc.dram_tensor("res", [HG, 128, RES_W], BF16, kind="ExternalInput")
    c15 = nc.dram_tensor("c15", [128, HG], F32, kind="ExternalInput")

    outp = nc.dram_tensor("outp", [N, C], F32, kind="ExternalOutput")
    k_out = nc.dram_tensor("k_out", [HG, D, M], F32R, kind="ExternalOutput")
    v_out = nc.dram_tensor("v_out", [HG, M, D], F32R, kind="ExternalOutput")

    with tile.TileContext(nc) as tc, ExitStack() as ctx:
        keep = ctx.enter_context(tc.tile_pool(name="keep", bufs=1))
        qh_sb = keep.tile([128, 4, N], BF16)  # qhT: head-pair blocks x n
        kT_sb = keep.tile([128, 4, M], F32R)  # kT: head-pair blocks x m
        vA_sb = keep.tile([128, 8, HG, D + 1], F32R)  # v (+ ones col)
        kb_sb = keep.tile([128, 4, M], BF16)  # bf16 copy of kT for QK
        vb_sb = keep.tile([128, 8, HG, D + 1], BF16)  # bf16 v (+ ones) for AV
        ao_sb = keep.tile([128, 4, N], BF16)  # attn-out T (a x n)
        c15_sb = keep.tile([128, HG], F32)

        nc.sync.dma_start(out=c15_sb, in_=c15[:])
        nc.vector.memset(vA_sb[:, :, :, D].bitcast(F32), 1.0)
        nc.vector.memset(vb_sb[:, :, :, D], 1.0)

        # ---- Phase A: projections ----
        with ExitStack() as esA:
            pa = esA.enter_context(tc.tile_pool(name="pa", bufs=1))
            qT_sb = pa.tile([128, 8, N], BF16)
            kvT_sb = pa.tile([128, 8, M], F32R)
            wq_sb = pa.tile([128, 8, 512], BF16)
            wk_sb = pa.tile([128, 8, 512], F32R)
            wv_sb = pa.tile([128, 8, 512], F32R)
            pps = esA.enter_context(tc.tile_pool(name="pps", bufs=4, space="PSUM"))

            nc.sync.dma_start(out=wq_sb, in_=WqT.rearrange("(a p) o -> p a o", p=128))
            nc.sync.dma_start(out=wk_sb, in_=WkT.rearrange("(a p) o -> p a o", p=128))
            nc.sync.dma_start(out=wv_sb, in_=WvT.rearrange("(a p) o -> p a o", p=128))
            nc.sync.dma_start(out=qT_sb, in_=qT.rearrange("(a p) n -> p a n", p=128))
            nc.sync.dma_start(out=kvT_sb, in_=kvT.rearrange("(a p) n -> p a n", p=128))

            # qhT / kT: out[o, n] = W.T[o, c] @ xT[c, n]
            for x_sb, w_sb, o_sb in ((qT_sb, wq_sb, qh_sb), (kvT_sb, wk_sb, kT_sb)):
                for pb in range(4):
                    for nh in range(2):
                        ps = pps.tile([128, 512], F32, tag="pa_ps")
                        for ci in range(8):
                            nc.tensor.matmul(
                                ps,
                                (w_sb[:, ci, pb * 128 : (pb + 1) * 128]),
                                (x_sb[:, ci, nh * 512 : (nh + 1) * 512]),
                                start=(ci == 0),
                                stop=(ci == 7),
                            )
                        nc.scalar.copy(
                            out=o_sb[:, pb, nh * 512 : (nh + 1) * 512], in_=ps
                        )
                        if o_sb is kT_sb:
                            nc.vector.tensor_copy(
                                out=kb_sb[:, pb, nh * 512 : (nh + 1) * 512], in_=ps
                            )

            # v: out[m, o] = kv[m, c] @ Wv.T[c, o]
            for mb in range(8):
                ps = pps.tile([128, 512], F32, tag="pa_ps")
                for ci in range(8):
                    nc.tensor.matmul(
                        ps,
                        (kvT_sb[:, ci, mb * 128 : (mb + 1) * 128]),
                        (wv_sb[:, ci, :]),
                        start=(ci == 0),
                        stop=(ci == 7),
                    )
                nc.vector.tensor_copy(
                    out=vA_sb[:, mb, :, 0:D],
                    in_=ps[:].rearrange("p (h d) -> p h d", h=HG),
                )
                nc.vector.tensor_copy(
                    out=vb_sb[:, mb, :, 0:D],
                    in_=ps[:].rearrange("p (h d) -> p h d", h=HG),
                )
                nc.sync.dma_start(
                    out=v_out[:, mb * 128 : (mb + 1) * 128, :].rearrange(
                        "h m d -> m h d"
                    ),
                    in_=vA_sb[:, mb, :, 0:D],
                )

            for pb in range(4):
                nc.sync.dma_start(
                    out=k_out[2 * pb : 2 * pb + 2].rearrange("t d m -> (t d) m"),
                    in_=kT_sb[:, pb, :],
                )

        # ---- Phase B: attention ----
        with ExitStack() as esB:
            pbp = esB.enter_context(tc.tile_pool(name="pbp", bufs=1))
            wp_sb = pbp.tile([128, 4, C], BF16)
            nc.sync.dma_start(out=wp_sb, in_=WpT.rearrange("(a p) o -> p a o", p=128))
            res_pool = esB.enter_context(tc.tile_pool(name="res", bufs=3))
            pt_pool = esB.enter_context(tc.tile_pool(name="pt", bufs=3))
            nrm_pool = esB.enter_context(tc.tile_pool(name="nrm", bufs=4))
            dbounce = esB.enter_context(tc.tile_pool(name="dbounce", bufs=4, space="DRAM"))

            with ExitStack() as esB2:
                sc_pool = esB2.enter_context(
                    tc.tile_pool(name="sc", bufs=2, space="PSUM")
                )
                o_pool = esB2.enter_context(
                    tc.tile_pool(name="op", bufs=4, space="PSUM")
                )

                for pr in range(4):
                    res_t = []
                    for hi in range(2):
                        rt = res_pool.tile([128, RES_W], BF16, tag="res")
                        nc.sync.dma_start(out=rt, in_=res[2 * pr + hi])
                        res_t.append(rt)
                    ops = {
                        (hi, nh): o_pool.tile(
                            [65, 512], F32, tag="o", name=f"o_{pr}_{hi}_{nh}"
                        )
                        for hi in range(2)
                        for nh in range(2)
                    }
                    for mi in range(8):
                        m0 = mi * 128
                        for hi in range(2):
                            h = 2 * pr + hi
                            st = sc_pool.tile([128, 1024], F32, tag="sc")
                            for nh in range(2):
                                nc.tensor.matmul(
                                    st[:, nh * 512 : (nh + 1) * 512],
                                    (kb_sb[hi * 64 : hi * 64 + 64, pr, m0 : m0 + 128]),
                                    (
                                        qh_sb[
                                            hi * 64 : hi * 64 + 64,
                                            pr,
                                            nh * 512 : (nh + 1) * 512,
                                        ]
                                    ),
                                    start=True,
                                    stop=True,
                                    tile_position=(hi * 64, 0),
                                )
                            L = min(m0 + 256, 1024)
                            nc.vector.tensor_add(
                                st[:, 0:L],
                                st[:, 0:L],
                                res_t[hi][:, 896 - m0 : 896 - m0 + L],
                            )
                            pt = pt_pool.tile([128, 1024], BF16, tag="pt")
                            nc.scalar.activation(
                                out=pt,
                                in_=st[:],
                                func=EXP,
                                bias=c15_sb[:, h : h + 1],
                                scale=0.125,
                            )
                            for nh in range(2):
                                nc.tensor.matmul(
                                    ops[(hi, nh)],
                                    (vb_sb[:, mi, h, :]),
                                    (pt[:, nh * 512 : (nh + 1) * 512]),
                                    start=(mi == 0),
                                    stop=(mi == 7),
                                    skip_group_check=True,
                                )
                    # normalize: rows 0..63 = sum(exp * v), row 64 = sum(exp)
                    for hi in range(2):
                        for nh in range(2):
                            op = ops[(hi, nh)]
                            bc = nrm_pool.tile([65, 512], F32, tag="bc")
                            nc.scalar.activation(
                                out=bc[64:65, :], in_=op[64:65, :],
                                func=mybir.ActivationFunctionType.Ln,
                            )
                            nc.scalar.activation(
                                out=bc[64:65, :], in_=bc[64:65, :],
                                func=EXP, scale=-1.0,
                            )
                            db = dbounce.tile([1, 512], F32, tag="db")
                            nc.sync.dma_start(out=db, in_=bc[64:65, :])
                            nc.sync.dma_start(
                                out=bc[0:64, :], in_=db.to_broadcast((64, 512))
                            )
                            if hi == 0:
                                nc.vector.tensor_mul(
                                    ao_sb[0:64, pr, nh * 512 : (nh + 1) * 512],
                                    op[0:64, :],
                                    bc[0:64, :],
                                )
                            else:
                                sr = nrm_pool.tile([64, 512], BF16, tag="sr")
                                nc.vector.tensor_mul(sr, op[0:64, :], bc[0:64, :])
                                nc.sync.dma_start(
                                    out=ao_sb[64:128, pr, nh * 512 : (nh + 1) * 512],
                                    in_=sr,
                                )

            # ---- Phase C: output projection (partial; host adds halves+bias)
            with ExitStack() as esC:
                pcs = esC.enter_context(tc.tile_pool(name="pcs", bufs=2, space="PSUM"))
                out_pool = esC.enter_context(tc.tile_pool(name="out", bufs=2))
                for ni in range(8):
                    ot = out_pool.tile([128, 1024], F32, tag="ot")
                    for oh in range(2):
                        ps = pcs.tile([128, 512], F32, tag="cps")
                        for ab in range(4):
                            nc.tensor.matmul(
                                ps,
                                (ao_sb[:, ab, ni * 128 : (ni + 1) * 128]),
                                (wp_sb[:, ab, oh * 512 : (oh + 1) * 512]),
                                start=(ab == 0),
                                stop=(ab == 3),
                            )
                        nc.scalar.copy(out=ot[:, oh * 512 : (oh + 1) * 512], in_=ps)
                    nc.sync.dma_start(out=outp[ni * 128 : (ni + 1) * 128, :], in_=ot)

    nc.compile()
    return nc


def _round_f32r(x):
    """Round f32 to the fp32r (TF32-like, 11-bit mantissa) grid, matching
    walrus's fp32_to_fp32r (round-half-up at bit 12)."""
    u = np.ascontiguousarray(x, np.float32).view(np.uint32)
    return ((u + np.uint32(0x800)) & np.uint32(0xFFFFF000)).view(np.float32)


def _bucket_np(r):
    """numpy mirror of reference.relative_position_bucket (f32 arithmetic)."""
    nb = NUM_BUCKETS // 2  # 16
    rb = (r > 0).astype(np.int32) * nb
    ar = np.abs(r)
    me = nb // 2  # 8
    is_small = ar < me
    rf = np.maximum(ar, 1).astype(np.float32)
    large = me + (
        np.log(rf / np.float32(me))
        / np.float32(math.log(MAX_DISTANCE / me))
        * np.float32(nb - me)
    ).astype(np.int32)
    large = np.minimum(large, nb - 1)
    return rb + np.where(is_small, ar.astype(np.int32), large)


_CACHE = {}


def _program():
    if "nc" not in _CACHE:
        _CACHE["nc"] = _build_program()
    return _CACHE["nc"]


def _host_bias_tables(bias_table):
    """res[g][h, p, rc] = 8*(table[bucket(p + 896 - rc), H0+h] - c15) as bf16,
    plus c15 rows [128, HG] per group."""
    import ml_dtypes

    p = np.arange(128, dtype=np.int64)[:, None]
    rc = np.arange(RES_W, dtype=np.int64)[None, :]
    bkt = _bucket_np(p + 896 - rc)  # [128, RES_W]
    res_g, c15_g = [], []
    for g in range(2):
        H0 = g * HG
        tab = bias_table[:, H0 : H0 + HG].astype(np.float32)  # [32, HG]
        c15 = tab[15]  # [HG]
        r = 8.0 * (tab[bkt] - c15[None, None, :])  # [128, RES_W, HG]
        res_g.append(
            np.ascontiguousarray(r.transpose(2, 0, 1)).astype(ml_dtypes.bfloat16)
        )
        c15_g.append(np.ascontiguousarray(np.broadcast_to(c15, (128, HG))).astype(np.float32))
    return res_g, c15_g


def _to_bf16(x):
    import ml_dtypes

    return np.ascontiguousarray(x, np.float32).astype(ml_dtypes.bfloat16)


def kernel(q, kv, Wq, Wkv, Wproj, bproj, bias_table):
    q = np.asarray(q, np.float32)
    kv = np.asarray(kv, np.float32)
    Wq = np.asarray(Wq, np.float32)
    Wkv = np.asarray(Wkv, np.float32)
    Wproj = np.asarray(Wproj, np.float32)
    bproj = np.asarray(bproj, np.float32)
    bias_table = np.asarray(bias_table, np.float32)

    nc = _program()
    res_g, c15_g = _host_bias_tables(bias_table)

    wq_g, wk_g, wv_g, wp_g = [], [], [], []
    for g in range(2):
        s = slice(512 * g, 512 * (g + 1))
        wq_g.append(_to_bf16(Wq[s, :].T))
        wk_g.append(_round_f32r(Wkv[s, :].T))
        wv_g.append(_round_f32r(Wkv[1024 + 512 * g : 1024 + 512 * (g + 1), :].T))
        wp_g.append(_to_bf16(Wproj[:, s].T))

    in_maps = []
    for core in range(NCORES):
        b, g = core // 2, core % 2
        in_maps.append(
            {
                "qT": _to_bf16(q[b].T),
                "kvT": _round_f32r(kv[b].T),
                "WqT": wq_g[g],
                "WkT": wk_g[g],
                "WvT": wv_g[g],
                "WpT": wp_g[g],
                "res": res_g[g],
                "c15": c15_g[g],
            }
        )

    results = run_bass_kernel_spmd(
        nc, in_maps, core_ids=list(range(NCORES)), **_CACHE.get("run_kwargs", {})
    )
    _CACHE["last_results"] = results

    out = np.zeros((B, N, C), np.float32)
    key = np.empty((B, H, M, D), np.float32)
    value = np.empty((B, H, M, D), np.float32)
    for core, r in enumerate(results.results):
        b, g = core // 2, core % 2
        H0 = g * HG
        out[b] += r["outp"]
        key[b, H0 : H0 + HG] = np.asarray(r["k_out"]).transpose(0, 2, 1)
        value[b, H0 : H0 + HG] = np.asarray(r["v_out"])
    out += bproj[None, None, :]
    return out, key, value
